# revision 26
# baseline (speedup 1.0000x reference)
"""Multi-head attention TRN2 Bass kernel (v3).

Sharding: head-parallel across 8 cores (2 heads each). Each core computes
its heads' contribution through the row-sharded W_O matmul; the host sums
the 8 partial (N, D_MODEL) outputs (f16) and adds the bias terms.

v3 over v2:
  - Fine-grained input streaming: K half0 lands first (sync HWDGE), Q
    chunk0 rides the scalar queue behind the weights, V halves go on the
    idle gpsimd SWDGE queue. Projections chase the DMAs k-slice by
    k-slice, so the first exp fires at ~11us instead of ~29us.
  - Exp split across engines: ScalarE ACT (exact, bias=-20) for most
    (pair, head) score blocks; for D pairs per chunk the h1 block is
    computed on the otherwise-lighter DVE as a one-pass Schraudolph
    fast-exp: int16(round(A16*x + B16)) == the bf16 BITS of exp(x-20)
    (max rel err ~3%, washes out through the softmax ratio; verified
    end-to-end in numpy: rel_fro 1.41e-2 vs 1.33e-2 all-exact).
  - Last-chunk tail casts/norm-muls split between ScalarE (idle after
    the final exp) and DVE.

Per-core dataflow (all matmul inputs bf16, fp32 PSUM accumulation):
  KhT (128 x M)  = [Wk_h0|Wk_h1].T @ K.T   (head h in partitions 64h..64h+63)
  QhT (128 x N)  = same for Q
  Vh  (m x 128)  = V.T_tile.T @ Wv          (data-stationary)
  per n-chunk of 512, per m-tile pair:
    ST_h (m x n) = KhT_h[:, mtile].T @ QhT_h[:, chunk]  (two heads run
                   concurrently via tile_position row packing)
    E_h = exp(ST_h - 20)   ScalarE ACT or DVE fast-exp per schedule
  per n-tile of 128 (after all E of the chunk):
    U_h (n x 65) += E_h_tile.T @ [Vh_h | ones]
    G   (n x 128) = U[:, :64] * recip(U[:, 64])
    GT  (128 x n) = PE-transpose(G)
    partial (n x D_MODEL) = GT.T @ (dk^-0.5 * Wo_c)  -> f16 -> DRAM

Host: out = sum_c partial_c + dk^-0.5 * (bv_flat @ Wo) + bo
"""

import math
import os
from contextlib import ExitStack

import ml_dtypes
import numpy as np

N, M, D_MODEL, H, D_K, D_V = 2048, 2048, 1024, 16, 64, 64
NCORES = 8
HPC = H // NCORES        # heads per core = 2
DH = HPC * D_K           # 128 = stacked head dim per core
CH = 512                 # n-chunk (matmul moving free size)
NCH = N // CH            # 4
KT = D_MODEL // 128      # 8 contraction tiles for projections
MT = M // 128            # 16 m tiles
W2 = 2 * (D_V + 1)       # 130: per-mt vh_aug block: [Vh0|1|Vh1|1]
EXP_BIAS = -20.0         # constant shift, cancels in softmax; guards overflow

# Schraudolph fast-exp constants: int16(A16*x + B16) = bf16 bits of
# ~exp(x + EXP_BIAS) for x-EXP_BIAS in (-80, +100).  B16 splits the
# difference between truncate / round-to-nearest f32->i16 conversion.
A16 = (2.0 ** 23) / math.log(2.0) / 65536.0
B16 = (127.0 * 128.0 - 5.25) + A16 * EXP_BIAS

# which pairs' h1 score block runs on the DVE, per chunk
DVE_PAIRS = {
    0: (0, 1, 2, 3, 4, 5),
    1: (3, 4, 5, 6),
    2: (4, 5, 6),
    3: (4, 5, 6, 7),
}

_compiled = {}
LAST_RESULT = {}


def _build_bass():
    import concourse.tile as tile
    from concourse import bacc, mybir
    from concourse.masks import make_identity

    f32 = mybir.dt.float32
    f16 = mybir.dt.float16
    bf16 = mybir.dt.bfloat16
    i16 = mybir.dt.int16
    nc = bacc.Bacc(
        "TRN2",
        target_bir_lowering=False,
        debug=False,
        enable_asserts=False,
        num_devices=NCORES,
    )

    qt = nc.dram_tensor("qt", (D_MODEL, N), bf16, kind="ExternalInput").ap()
    kti = nc.dram_tensor("kt", (D_MODEL, M), bf16, kind="ExternalInput").ap()
    vti = nc.dram_tensor("vt", (D_MODEL, M), bf16, kind="ExternalInput").ap()
    # host pre-swizzles projection weights into SBUF layout (128, KT*DH)
    wq = nc.dram_tensor("wq", (128, KT * DH), bf16, kind="ExternalInput").ap()
    wk = nc.dram_tensor("wk", (128, KT * DH), bf16, kind="ExternalInput").ap()
    wv = nc.dram_tensor("wv", (128, KT * DH), bf16, kind="ExternalInput").ap()
    wo = nc.dram_tensor("wo", (DH, D_MODEL), bf16, kind="ExternalInput").ap()
    bq = nc.dram_tensor("bq", (DH, 1), f32, kind="ExternalInput").ap()
    bk = nc.dram_tensor("bk", (DH, 1), f32, kind="ExternalInput").ap()
    out = nc.dram_tensor("out", (N, D_MODEL), f16, kind="ExternalOutput").ap()

    Exp = mybir.ActivationFunctionType.Exp
    mul_op = mybir.AluOpType.mult
    add_op = mybir.AluOpType.add

    with tile.TileContext(nc) as tc, ExitStack() as ctx:
        cpool = ctx.enter_context(tc.tile_pool(name="const", bufs=1))

        wq_sb = cpool.tile([128, D_MODEL], bf16, tag="wq")
        wk_sb = cpool.tile([128, D_MODEL], bf16, tag="wk")
        wv_sb = cpool.tile([128, D_MODEL], bf16, tag="wv")
        wo_sb = cpool.tile([128, D_MODEL], bf16, tag="wo")
        bq_sb = cpool.tile([DH, 1], f32, tag="bq")
        bk_sb = cpool.tile([DH, 1], f32, tag="bk")
        id_sb = cpool.tile([128, 128], bf16, tag="id")
        eb_sb = cpool.tile([128, 1], f32, tag="eb")
        qht = cpool.tile([DH, N], bf16, tag="qht")
        kht = cpool.tile([DH, M], bf16, tag="kht")
        vh_aug = cpool.tile([128, MT * W2], bf16, tag="vaug")
        # full transposed inputs staged in SBUF
        qts = cpool.tile([128, KT * N], bf16, tag="qts")
        kts = cpool.tile([128, KT * M], bf16, tag="kts")
        vts = cpool.tile([128, KT * M], bf16, tag="vts")

        HM = M // 2
        # ALL input DMAs on the sync queue in strict priority order —
        # per-core DMA bandwidth (~340GB/s) is shared per-packet across
        # active queues, so splitting queues only dilutes the priority of
        # the first-needed data.  Order: weights, K half0, Q chunk0,
        # K half1, V half0, Q chunk1, V half1, Q chunks 2-3.
        nc.sync.dma_start(wk_sb[:], wk[:, :])
        nc.sync.dma_start(bk_sb[:], bk[:, :])
        nc.sync.dma_start(wq_sb[:], wq[:, :])
        nc.sync.dma_start(bq_sb[:], bq[:, :])
        for k in range(KT):
            nc.sync.dma_start(kts[:, k * M:k * M + HM], kti[k * 128:(k + 1) * 128, 0:HM])
        for k in range(KT):
            nc.sync.dma_start(qts[:, k * N:k * N + CH], qt[k * 128:(k + 1) * 128, 0:CH])
        nc.scalar.dma_start(wv_sb[:], wv[:, :])
        nc.scalar.dma_start(wo_sb[:], wo[:, :])
        for k in range(KT):
            nc.sync.dma_start(kts[:, k * M + HM:(k + 1) * M], kti[k * 128:(k + 1) * 128, HM:M])
        for k in range(KT):
            nc.sync.dma_start(vts[:, k * M:k * M + HM], vti[k * 128:(k + 1) * 128, 0:HM])
        for k in range(KT):
            nc.sync.dma_start(qts[:, k * N + CH:k * N + 2 * CH],
                              qt[k * 128:(k + 1) * 128, CH:2 * CH])
        for k in range(KT):
            nc.sync.dma_start(vts[:, k * M + HM:(k + 1) * M], vti[k * 128:(k + 1) * 128, HM:M])
        for c in range(2, NCH):
            for k in range(KT):
                nc.sync.dma_start(
                    qts[:, k * N + c * CH:k * N + (c + 1) * CH],
                    qt[k * 128:(k + 1) * 128, c * CH:(c + 1) * CH],
                )

        make_identity(nc, id_sb[:])
        nc.gpsimd.memset(vh_aug[:], 1.0)
        nc.gpsimd.memset(eb_sb[:], EXP_BIAS)

        with tc.tile_pool(name="ps", bufs=1, space="PSUM") as pp, \
                tc.tile_pool(name="wk2", bufs=2) as wpool:

            # PE warm-up: the HAM clock gate defaults to 1.2GHz and needs
            # ~3.4us of sustained activity to release to 2.4GHz. Also warm
            # the exp ACT table (one-time ~2.7us load) during the DMA ramp.
            warm = pp.tile([128, 2 * CH], f32, tag="st0", bufs=1, name="warm")
            scr = wpool.tile([128, 1], f32, tag="scr", bufs=1)
            nc.scalar.activation(scr[:], eb_sb[:], Exp)
            for i in range(30):
                nc.tensor.matmul(warm[:, 0:128], id_sb[:], id_sb[:],
                                 start=True, stop=True, skip_group_check=True)

            def proj_pass(x_sb, w_sb, out_sb, bias_sb, chunks, weave=0):
                # k-outer so matmuls chase the input DMAs slice by slice.
                # weave>0 inserts idle warm matmuls after each k-slice so the
                # HAM clock gate stays open through the DMA-chase gaps.
                ts = {}
                for ch in chunks:
                    ts[ch] = pp.tile([128, CH], f32, tag="sc", bufs=2, name=f"pj{ch}")
                for k in range(KT):
                    for _ in range(weave):
                        nc.tensor.matmul(warm[:, 0:128], id_sb[:], id_sb[:],
                                         start=True, stop=True,
                                         skip_group_check=True)
                    for ch in chunks:
                        nc.tensor.matmul(
                            ts[ch][:],
                            w_sb[:, k * DH:(k + 1) * DH],
                            x_sb[:, k * N + ch * CH:k * N + (ch + 1) * CH],
                            start=(k == 0),
                            stop=(k == KT - 1),
                        )
                for ch in chunks:
                    nc.vector.tensor_scalar_add(
                        out_sb[:, ch * CH:(ch + 1) * CH], ts[ch][:], bias_sb[:]
                    )

            def vproj_pass(g):
                # 4 m-tiles per pass, direct (m x dh) layout: data stationary.
                vp = pp.tile([128, CH], f32, tag="sc", bufs=2, name="vp")
                for i, mt in enumerate(g):
                    for k in range(KT):
                        nc.tensor.matmul(
                            vp[:, i * 128:(i + 1) * 128],
                            vts[:, k * M + mt * 128:k * M + (mt + 1) * 128],
                            wv_sb[:, k * DH:(k + 1) * DH],
                            start=(k == 0),
                            stop=(k == KT - 1),
                        )
                for i, mt in enumerate(g):
                    b = mt * W2
                    nc.vector.tensor_copy(vh_aug[:, b:b + D_V], vp[:, i * 128:i * 128 + D_V])
                    nc.vector.tensor_copy(
                        vh_aug[:, b + D_V + 1:b + W2 - 1],
                        vp[:, i * 128 + D_V:i * 128 + 2 * D_V],
                    )

            # ramp: only what the first score pairs need; the rest of the
            # projections drain into chunk 0's exp-wait gaps
            proj_pass(kts, wk_sb, kht, bk_sb, [0, 1], weave=2)
            proj_pass(qts, wq_sb, qht, bq_sb, [0], weave=3)

            def make_pv_emit(ex_pairs, u_state):
                # PV chain instructions for one mt-pair; chains for all
                # (h, nt) accumulate into 2 U banks. Only the globally-first
                # matmul into each bank uses start=True.
                def pv_emit(p):
                    if "A" not in u_state:
                        u_state["A"] = pp.tile([128, 4 * (D_V + 1)], f32,
                                               tag="u", bufs=2, name="uA")
                        u_state["B"] = pp.tile([128, 4 * (D_V + 1)], f32,
                                               tag="u", bufs=2, name="uB")
                        u_state["started"] = set()
                    for j in range(2):
                        mt = 2 * p + j
                        for h in range(HPC):
                            for nt in range(4):
                                key = "A" if nt < 2 else "B"
                                u = u_state[key]
                                off = (nt % 2) * 130 + h * 65
                                first = key not in u_state["started"]
                                u_state["started"].add(key)
                                last = (mt == MT - 1 and h == HPC - 1
                                        and nt % 2 == 1)
                                ex = ex_pairs[(h, p)]
                                eoff = j * CH + nt * 128
                                nc.tensor.matmul(
                                    u[:, off:off + 65],
                                    ex[:, eoff:eoff + 128],
                                    vh_aug[:, mt * W2 + h * 65:mt * W2 + h * 65 + 65],
                                    start=first,
                                    stop=last,
                                    skip_group_check=True,
                                )
                return pv_emit

            def build_tail(c, u_state):
                # post-chain per-chunk work: normalize, transpose, Wo, out.
                work = []
                g_tiles = {}
                last = c == NCH - 1

                def norm(nt):
                    def f():
                        u = u_state["A" if nt < 2 else "B"]
                        off = (nt % 2) * 130
                        g = wpool.tile([128, 128], bf16, tag="g", bufs=4, name=f"g{nt}")
                        g_tiles[nt] = g
                        for h in range(HPC):
                            rcp = wpool.tile([128, 1], f32, tag="rcp", bufs=6, name=f"rcp{nt}_{h}")
                            nc.vector.reciprocal(
                                rcp[:], u[:, off + h * 65 + D_V:off + h * 65 + D_V + 1])
                            if last and (nt + h) % 2 == 1:
                                # ScalarE is idle after the final exp
                                nc.scalar.mul(
                                    g[:, h * D_V:(h + 1) * D_V],
                                    u[:, off + h * 65:off + h * 65 + D_V], rcp[:]
                                )
                            else:
                                nc.vector.tensor_scalar_mul(
                                    g[:, h * D_V:(h + 1) * D_V],
                                    u[:, off + h * 65:off + h * 65 + D_V], rcp[:]
                                )
                    return f

                gt_tiles = {}

                def gtrans(nt):
                    def f():
                        gp = pp.tile([128, 128], bf16, tag="sc", bufs=2, name=f"gp{nt}")
                        gt = wpool.tile([128, 128], bf16, tag="gt", bufs=4, name=f"gt{nt}")
                        gt_tiles[nt] = gt
                        nc.tensor.transpose(gp[:], g_tiles[nt][:], id_sb[:])
                        nc.vector.tensor_copy(gt[:], gp[:])
                    return f

                ob_tiles = {}

                def wo_half(nt, half):
                    def f():
                        if half == 0:
                            ob_tiles[nt] = wpool.tile(
                                [128, D_MODEL], f16, tag="ob", bufs=6, name=f"ob{nt}")
                        ob = ob_tiles[nt]
                        wp = pp.tile([128, CH], f32, tag="sc", bufs=2, name=f"wp{nt}_{half}")
                        nc.tensor.matmul(
                            wp[:],
                            gt_tiles[nt][:],
                            wo_sb[:, half * CH:(half + 1) * CH],
                            start=True,
                            stop=True,
                        )
                        if last and (nt * 2 + half) % 2 == 1:
                            # split the f32->f16 casts across both engines
                            nc.scalar.copy(ob[:, half * CH:(half + 1) * CH], wp[:])
                        else:
                            nc.vector.tensor_copy(ob[:, half * CH:(half + 1) * CH], wp[:])
                        if half == 1:
                            n0 = c * CH + nt * 128
                            nc.sync.dma_start(out[n0:n0 + 128, :], ob[:])
                    return f

                for nt in range(4):
                    work.append((0.0, norm(nt)))
                for nt in range(4):
                    work.append((0.5, gtrans(nt)))
                    work.append((1.0, wo_half(nt, 0)))
                    work.append((1.0, wo_half(nt, 1)))
                return work

            tail = []
            for c in range(NCH):
                filler = list(tail)
                if c == 0:
                    # K half1 / V land during chunk 0; the deferred work
                    # drains at fixed pair slots matched to the DMA schedule
                    filler = [
                        (2.0, lambda: proj_pass(kts, wk_sb, kht, bk_sb, [2])),
                        (2.0, lambda: proj_pass(kts, wk_sb, kht, bk_sb, [3])),
                        (2.0, lambda: proj_pass(qts, wq_sb, qht, bq_sb, [1])),
                    ]
                    vwork = [lambda g=g: vproj_pass(g)
                             for g in ([0, 1, 2, 3], [4, 5, 6, 7],
                                       [8, 9, 10, 11], [12, 13, 14, 15])]
                else:
                    vwork = []

                ex_pairs = {}
                u_state = {}
                pv_emit = make_pv_emit(ex_pairs, u_state)
                npairs = MT // 2
                dve_pairs = DVE_PAIRS[c]
                for p in range(npairs):
                    st0 = pp.tile([128, 2 * CH], f32, tag="st0", bufs=1)
                    st1 = pp.tile([128, 2 * CH], f32, tag="st1", bufs=1)
                    for j in range(2):  # j: which mt of the pair
                        mt = 2 * p + j
                        nc.tensor.matmul(
                            st0[:, j * CH:(j + 1) * CH],
                            kht[0:64, mt * 128:(mt + 1) * 128],
                            qht[0:64, c * CH:(c + 1) * CH],
                            start=True, stop=True,
                            tile_position=(0, 0),
                        )
                        nc.tensor.matmul(
                            st1[:, j * CH:(j + 1) * CH],
                            kht[64:128, mt * 128:(mt + 1) * 128],
                            qht[64:128, c * CH:(c + 1) * CH],
                            start=True, stop=True,
                            tile_position=(64, 0),
                        )
                    ex0 = wpool.tile([128, 2 * CH], bf16, tag="ex", bufs=18)
                    nc.scalar.activation(ex0[:], st0[:], Exp, bias=eb_sb[:])
                    if p in dve_pairs:
                        # DVE Schraudolph: bf16 bits of exp(st - 20) via i16.
                        # The i16-bitcast write's WAR dep against earlier
                        # LDWEIGHTS readers of the recycled slot is not
                        # tracked; the strided memset (one col per 128-col
                        # PV slice) re-establishes it through a normal AP.
                        ex1 = wpool.tile([128, 2 * CH], bf16, tag="exd", bufs=10)
                        guard = ex1[:].rearrange("p (a b) -> p a b", b=128)[:, :, 0:1]
                        nc.vector.memset(guard, 0.0)
                        nc.vector.tensor_scalar(
                            ex1[:].bitcast(i16), st1[:], A16, B16, mul_op, add_op
                        )
                    else:
                        ex1 = wpool.tile([128, 2 * CH], bf16, tag="ex", bufs=18)
                        nc.scalar.activation(ex1[:], st1[:], Exp, bias=eb_sb[:])
                    ex_pairs[(0, p)] = ex0
                    ex_pairs[(1, p)] = ex1

                    if c == 0:
                        # chunk 0: K half1 / Q1 / V arrive mid-chunk; drain
                        # the deferred work at slots matched to DMA arrival
                        if p == 3 and filler:
                            filler.pop(0)[1]()      # kproj[2]
                        if p == 4 and filler:
                            filler.pop(0)[1]()      # kproj[3]
                        if p == 5 and filler:
                            filler.pop(0)[1]()      # qproj[1]
                        if p == 6 and vwork:
                            vwork.pop(0)()          # vgroup0
                        if p == 7:
                            if vwork:
                                vwork.pop(0)()      # vgroup1
                            pv_emit(0)
                            pv_emit(1)
                    else:
                        budget = 2.4
                        while budget > 0 and filler:
                            cost, f = filler.pop(0)
                            f()
                            budget -= cost
                        lag = 1 if c == NCH - 1 else 2
                        if p >= lag and (c == NCH - 1 or p - lag <= npairs - 4):
                            pv_emit(p - lag)
                if c == NCH - 1:
                    pv_emit(npairs - 1)
                g2_fn = vwork.pop(0) if vwork else None
                g3_fn = vwork.pop(0) if vwork else None
                while filler:
                    filler.pop(0)[1]()
                tail = build_tail(c, u_state)
                if c == 0:
                    # chunk 0 extras in dependency order: PV pairs deferred
                    # while V was loading (vgroups 2-3 woven in just before
                    # the pairs that need them), then Q proj chunks 2-3
                    pre = [(1.3, lambda pe=pv_emit: pe(2)),
                           (1.3, lambda pe=pv_emit: pe(3)),
                           (1.75, g2_fn),
                           (1.3, lambda pe=pv_emit: pe(4)),
                           (1.3, lambda pe=pv_emit: pe(5)),
                           (1.75, g3_fn),
                           (1.3, lambda pe=pv_emit: pe(6)),
                           (1.3, lambda pe=pv_emit: pe(7))]
                    tail = pre + tail
                    tail.insert(len(pre) + 4 + 2,
                                (2.0, lambda: proj_pass(qts, wq_sb, qht, bq_sb, [2])))
                    tail.insert(len(pre) + 4 + 5,
                                (2.0, lambda: proj_pass(qts, wq_sb, qht, bq_sb, [3])))
                elif c < NCH - 1:
                    # finish this chunk's last PV pairs inside the next
                    # chunk's drain instead of bunching them here
                    tail.insert(0, (1.3, lambda pe=pv_emit: pe(npairs - 3)))
                    tail.insert(1, (1.3, lambda pe=pv_emit: pe(npairs - 2)))
                    tail.insert(2, (1.3, lambda pe=pv_emit: pe(npairs - 1)))
            # last chunk: execute the tail stage-ordered so PE and DVE
            # ping-pong; dummy matmuls hold the HAM clock gate open
            warm2 = pp.tile([128, 2 * CH], f32, tag="st1", bufs=1, name="warm2")
            t = tail
            for j, idx in enumerate((0, 1, 2, 3, 4, 7, 5, 6, 8, 9, 10, 13, 11, 12, 14, 15)):
                t[idx][1]()
                if j >= 3:
                    nc.tensor.matmul(warm2[:, 0:128], id_sb[:], id_sb[:],
                                     start=True, stop=True, skip_group_check=True)

    nc.compile()
    return nc


def _get_nc():
    if "nc" not in _compiled:
        _compiled["nc"] = _build_bass()
    return _compiled["nc"]


def _ensure_ntff_hook():
    """Install the axon NTFF profile hook when the image's antenv lacks
    axon_hooks (trace support only; no-op when already present)."""
    import sys
    import types

    try:
        from antenv.axon_hooks import get_axon_ntff_profile_hook  # noqa: F401
        return
    except ImportError:
        pass
    try:
        import antenv
        from trn_agent_boot.trn_boot import _ntff_profile_via_ctypes

        so_path = "/opt/axon/libaxon_pjrt.so"
        if not os.path.exists(so_path):
            return
        hook = _ntff_profile_via_ctypes(so_path)
        mod = types.ModuleType("antenv.axon_hooks")
        state = {"hook": hook}
        mod.set_axon_ntff_profile_hook = lambda h: state.__setitem__("hook", h)
        mod.get_axon_ntff_profile_hook = lambda: state["hook"]
        sys.modules["antenv.axon_hooks"] = mod
        antenv.axon_hooks = mod
        import concourse.bass_utils as _bu

        _bu.upload_artifacts = lambda tmpdir: tmpdir
    except Exception as e:  # pragma: no cover - best effort
        print(f"ntff hook install failed: {e}")


def kernel(**inputs):
    from concourse.bass_utils import run_bass_kernel_spmd

    nc = _get_nc()
    bf = ml_dtypes.bfloat16
    Q = np.asarray(inputs["Q"], dtype=np.float32)
    K = np.asarray(inputs["K"], dtype=np.float32)
    V = np.asarray(inputs["V"], dtype=np.float32)
    Wq = np.asarray(inputs["Wq"], dtype=np.float32)
    bq = np.asarray(inputs["bq"], dtype=np.float32)
    Wk = np.asarray(inputs["Wk"], dtype=np.float32)
    bk = np.asarray(inputs["bk"], dtype=np.float32)
    Wv = np.asarray(inputs["Wv"], dtype=np.float32)
    Wo = np.asarray(inputs["Wo"], dtype=np.float32)
    bv = np.asarray(inputs["bv"], dtype=np.float32)
    bo = np.asarray(inputs["bo"], dtype=np.float32)
    scale = np.float32(D_K ** -0.5)

    qt = np.ascontiguousarray(Q.T).astype(bf)
    kt = np.ascontiguousarray(K.T).astype(bf)
    vt = np.ascontiguousarray(V.T).astype(bf)

    def swz(w):  # (D_MODEL, DH) -> SBUF layout (128, KT*DH)
        return np.ascontiguousarray(
            w.reshape(KT, 128, DH).transpose(1, 0, 2).reshape(128, KT * DH)
        )

    in_maps = []
    for c in range(NCORES):
        h0 = HPC * c
        hs = list(range(h0, h0 + HPC))
        in_maps.append(
            dict(
                qt=qt,
                kt=kt,
                vt=vt,
                wq=swz(np.concatenate([Wq[h] for h in hs], axis=1)).astype(bf),
                wk=swz(np.concatenate([Wk[h] for h in hs], axis=1)).astype(bf),
                wv=swz(np.concatenate([Wv[h] for h in hs], axis=1)).astype(bf),
                wo=np.ascontiguousarray(Wo[h0 * D_V:(h0 + HPC) * D_V, :] * scale).astype(bf),
                bq=np.ascontiguousarray(bq[h0:h0 + HPC].reshape(DH, 1)),
                bk=np.ascontiguousarray(bk[h0:h0 + HPC].reshape(DH, 1)),
            )
        )

    trace = bool(int(os.environ.get("BASS_KERNEL_TRACE", "0")))
    if trace:
        _ensure_ntff_hook()
        tmpdir = os.environ.get("BASS_KERNEL_TMPDIR")
        res = run_bass_kernel_spmd(
            nc, in_maps, list(range(NCORES)), trace=True, tmpdir=tmpdir
        )
    else:
        res = run_bass_kernel_spmd(nc, in_maps, list(range(NCORES)))
    LAST_RESULT["exec_time_ns"] = res.exec_time_ns
    LAST_RESULT["res"] = res

    Y = np.zeros((N, D_MODEL), np.float32)
    for c in range(NCORES):
        Y += np.asarray(res.results[c]["out"], dtype=np.float32)
    Y += scale * (bv.reshape(-1) @ Wo) + bo
    return Y
